# revision 34
# baseline (speedup 1.0000x reference)
"""Multi-head attention (B=4, S=2048, H=1024, 16 heads) on 8 trn2 NeuronCores.

Sharding: data-parallel over batch (4) x tensor-parallel over head-groups (2):
core c handles batch c//2, heads 8*(c%2) .. 8*(c%2)+8. Each core computes its
partial output projection; host sums the two head-group partials + bo.

Per-core device algorithm (all matmul inputs bf16, fp32 accumulation):
  inputs are pre-transposed/packed on host: xqp/xkp hold x^T slices laid out
  so each (t,s) projection needs ONE [128,4096] DMA; wqk packs the per-
  head-pair Q and K weight slices into one [128,2048] DMA per phase.
  QT[t] (128=2 heads' d, sq) = wqk-slices^T @ x-chunks (+bias)
  KT[t][s'] likewise (split per sk-chunk for fine-grained deps)
  V halves (pairs 01 / 23), per 128-sk tile: [values(64)|ones(64)] per head;
  the ones columns make the ctx matmul emit the softmax denominator
  replicated in psum rows 64:128 at zero PE cost (matmul time ~ N only).
  per head-pair t, sq-chunk s (512), sk-tile i (128):
     S^T = KT-slice^T @ QT-slice  (two heads row-packed, run concurrently)
     P^T = exp(S^T * 0.125)       (ACT, psum->sbuf, bf16 out)
     ctx (128, 512) += V-block^T @ P^T
  normalize: ctx psum -> sbuf copy, reciprocal of rows 64:128 (DVE),
     CX = ctx * recip (gpsimd, to keep DVE short)
  out (sq, 1024) = sum_t CX-chunks^T @ woT   -> DMA out (fp32)

Schedule: ACT (exp) is the hard floor (256 x ~1.15us). Phase t=0 pays the
V-pair-01 projection jit inside chunk (0,0) and interleaves K(0,s')
projections with DMA arrival; V pairs 23 and the t+1 Q/K projections hide
in the ACT-bound phases' PE slack; the output projection of s-chunk j is
emitted at the end of chunk (3, j+1) so the in-order PE stream reaches it
only after the DVE epilogue that finalizes CX has long finished.
"""
import os
import sys

sys.path.insert(0, "/opt/trn_rl_repo")

import numpy as np
import ml_dtypes

import concourse.bass as bass
import concourse.mybir as mybir
import concourse.tile as tile

# ---------------------------------------------------------------------------
# Walrus in this environment allows at most 1 sync wait per instruction (2 for
# EventSemaphore); Tile sometimes emits more (e.g. the exit drain). Hoist the
# extra waits onto EventSemaphore instructions inserted before the offender.
import json as _json


def _transform_bir_json(bir_bytes: bytes) -> bytes:
    bir = _json.loads(bir_bytes)
    changed = False
    ctr = 0
    for fn in bir.get("functions", []):
        for blk in fn.get("blocks", []):
            out = []
            for inst in blk.get("instructions", []):
                si = inst.get("sync_info") or {}
                waits = si.get("on_wait") or []
                cap = 2 if inst.get("opcode") == "EventSemaphore" else 1
                if len(waits) > cap:
                    changed = True
                    extra = waits[:-cap]
                    si["on_wait"] = waits[-cap:]
                    for i in range(0, len(extra), 2):
                        ctr += 1
                        out.append(
                            {
                                "debug": inst.get("debug"),
                                "engine": inst["engine"],
                                "ins": [],
                                "name": f"{inst['name']}_xw{ctr}",
                                "opcode": "EventSemaphore",
                                "outs": [],
                                "sync_info": {
                                    "on_update": [],
                                    "on_wait": extra[i : i + 2],
                                },
                            }
                        )
                out.append(inst)
            blk["instructions"] = out
    if not changed:
        return bir_bytes
    return _json.dumps(bir).encode()


def _apply_bir_patch():
    import concourse.bass_utils as bu
    import concourse.bass2jax as b2j

    if getattr(b2j, "_bir_waitfix_applied", False):
        return
    orig = bu.compile_bir_kernel

    def patched(bir_json, tmpdir, neff_name="file.neff"):
        return orig(_transform_bir_json(bir_json), tmpdir, neff_name)

    b2j.compile_bir_kernel = patched
    bu.compile_bir_kernel = patched
    b2j._bir_waitfix_applied = True


_apply_bir_patch()

from concourse.bass_utils import run_bass_kernel_spmd  # noqa: E402

# ---------------------------------------------------------------------------
HIDDEN = 1024
HEADS = 16
HD = 64  # head dim
B, SQ, SK = 4, 2048, 2048
NCORES = 8
HPC = 8  # heads per core (tensor-parallel over 2 head groups)
HL = HPC * HD  # local hidden slice = 512
SCALE = HD ** -0.5

F32 = mybir.dt.float32
BF16 = mybir.dt.bfloat16

_CACHED = {}


def _build_nc(dt_mm):
    nc = bass.Bass()
    xqp_d = nc.declare_dram_parameter("xqp", [128, 4 * 4096], dt_mm, isOutput=False)
    xkp_d = nc.declare_dram_parameter("xkp", [128, 4 * 4096], dt_mm, isOutput=False)
    xvT_d = nc.declare_dram_parameter("xvT", [HIDDEN, SK], dt_mm, isOutput=False)
    wqk_d = nc.declare_dram_parameter("wqk", [128, 4 * 2048], dt_mm, isOutput=False)
    wvp_d = nc.declare_dram_parameter("wvp", [128, 4096], dt_mm, isOutput=False)
    wop_d = nc.declare_dram_parameter("wop", [128, 4096], dt_mm, isOutput=False)
    bq_d = nc.declare_dram_parameter("bq2", [128, 4], F32, isOutput=False)
    bk_d = nc.declare_dram_parameter("bk2", [128, 4], F32, isOutput=False)
    bvb_d = nc.declare_dram_parameter("bvb", [128, HL], F32, isOutput=False)
    out_d = nc.declare_dram_parameter("out", [SQ, HIDDEN], F32, isOutput=True)

    NHC = HIDDEN // 128  # 8 hidden chunks
    NT = 4  # head-pair tiles (8 local heads -> 4 pairs of 64 rows)
    NS = 4  # sq chunks of 512
    NI = SK // 128  # 16 sk tiles

    with tile.TileContext(nc) as tc:
        from contextlib import ExitStack

        with ExitStack() as stack:
            wpool = stack.enter_context(tc.tile_pool(name="wpool", bufs=1))
            apool = stack.enter_context(tc.tile_pool(name="apool", bufs=1))

            # ---- persistent weights / biases (DMAs emitted at point of need)
            # Q and K weight slices as separate tiles so the very first Q
            # projection doesn't wait on the K half of the transfer.
            wqk_sb = [
                [
                    wpool.tile([128, 1024], dt_mm, name=f"w{qk}{t}", tag=f"w{qk}{t}")
                    for qk in ("q", "k")
                ]
                for t in range(NT)
            ]
            wv_sb = wpool.tile([128, 4096], dt_mm, name="wvp", tag="wvp")
            wo_sb = wpool.tile([128, 4096], dt_mm, name="wop", tag="wop")
            bq_sb = wpool.tile([128, 4], F32)
            bk_sb = wpool.tile([128, 4], F32)
            bvb_sb = wpool.tile([128, HL], F32)

            # ---- persistent activations. All split per (t, s-chunk) so
            # every dependency is tile-granular: projections, epilogues and
            # deferred work can interleave into the attention chunks without
            # false write-during-read hazards.
            QT = [
                [
                    apool.tile([128, 512], dt_mm, name=f"QT{t}_{s}", tag=f"QT{t}_{s}")
                    for s in range(NS)
                ]
                for t in range(NT)
            ]
            KT = [
                [
                    apool.tile([128, 512], dt_mm, name=f"KT{t}_{c}", tag=f"KT{t}_{c}")
                    for c in range(NS)
                ]
                for t in range(NT)
            ]
            # V in two halves (head pairs 01 / 23), [128, 512] each:
            # per head a 128-col block [0:64]=values, [64:128]=ones.
            VH = [
                [
                    apool.tile([128, 512], dt_mm, name=f"V{h}_{i}", tag=f"V{h}_{i}")
                    for i in range(NI)
                ]
                for h in range(2)
            ]
            CX = [
                [
                    apool.tile([128, 512], dt_mm, name=f"CX{t}_{s}", tag=f"CX{t}_{s}")
                    for s in range(NS)
                ]
                for t in range(NT)
            ]

            inner = stack.enter_context(ExitStack())
            spool = inner.enter_context(tc.tile_pool(name="ldpool", bufs=2))
            dpool = inner.enter_context(tc.tile_pool(name="dpool", bufs=4))
            psA = inner.enter_context(tc.tile_pool(name="psA", bufs=2, space="PSUM"))
            psS = inner.enter_context(tc.tile_pool(name="psS", bufs=2, space="PSUM"))
            psC = inner.enter_context(tc.tile_pool(name="psC", bufs=1, space="PSUM"))

            def emit_x_dma(which, t, s):
                xp_d, off, b_sb, OUT, nm = which
                xch = spool.tile(
                    [128, 4096], dt_mm, name=f"x{nm}{t}{s}", tag="xch", bufs=3
                )
                nc.sync.dma_start(
                    out=xch[:], in_=xp_d[:, 4096 * s : 4096 * s + 4096]
                )
                return xch

            def emit_proj(t, s, which, xch=None):
                xp_d, off, b_sb, OUT, nm = which
                if xch is None:
                    xch = emit_x_dma(which, t, s)
                ps = psA.tile([128, 512], F32, name=f"ps{nm}{s}{t}", tag="psA")
                w = wqk_sb[t][0 if off == 0 else 1]
                for c in range(NHC):
                    nc.tensor.matmul(
                        ps[:],
                        w[:, 128 * c : 128 * c + 128],
                        xch[:, 512 * c : 512 * c + 512],
                        start=(c == 0),
                        stop=(c == NHC - 1),
                    )
                nc.vector.tensor_scalar_add(OUT[t][s][:, :], ps[:], b_sb[:, t : t + 1])

            def emit_v_tile(h, i):
                # V half h (head pairs 2h,2h+1), sk-tile i: 8 accumulating
                # matmuls of N=256 into half a psA slot, then ones + bias.
                # xv is split in sk-halves so the i<8 tiles only wait on the
                # first half of the (late-arriving) xv DMA stream.
                xvh = xv_sb[i // 8]
                io = 128 * (i % 8)
                ps = psA.tile([128, 512], F32, name=f"psv{h}_{i}", tag="psA")
                for c in range(NHC):
                    nc.tensor.matmul(
                        ps[:, 0:256],
                        xvh[c][:, io : io + 128],
                        wv_sb[:, 512 * c + 256 * h : 512 * c + 256 * h + 256],
                        start=(c == 0),
                        stop=(c == NHC - 1),
                    )
                nc.gpsimd.memset(VH[h][i][:], 1.0)
                vv = VH[h][i].rearrange("p (h e) -> p h e", e=128)
                nc.vector.tensor_add(
                    vv[:, :, 0:HD],
                    ps[:, 0:256].rearrange("p (h d) -> p h d", d=HD),
                    bvb_sb[:, 256 * h : 256 * h + 256].rearrange("p (h d) -> p h d", d=HD),
                )

            def emit_outproj(q):
                # output projection for one finished q-tile; reuses the psA
                # slots that the (by now finished) projections vacated.
                qs, qo = q // 4, 128 * (q % 4)
                ot = dpool.tile([128, HIDDEN], F32, name=f"ot{q}", tag="ot", bufs=2)
                for half in range(2):
                    po = psA.tile([128, 512], F32, name=f"po{q}_{half}", tag="psA")
                    for tt in range(NT):
                        nc.tensor.matmul(
                            po[:],
                            CX[tt][qs][:, qo : qo + 128],
                            wo_sb[:, 1024 * tt + 512 * half : 1024 * tt + 512 * half + 512],
                            start=(tt == 0),
                            stop=(tt == NT - 1),
                        )
                    nc.vector.tensor_copy(ot[:, 512 * half : 512 * half + 512], po[:])
                nc.sync.dma_start(out=out_d[128 * q : 128 * q + 128, :], in_=ot[:])

            def emit_attention_chunk(t, s, hooks=None, final=False):
                qt_lo, qt_hi = QT[t][s][0:64, :], QT[t][s][64:128, :]
                Vh = VH[t // 2]
                vb = 256 * (t % 2)  # pair block offset inside the half
                ctx0 = psC.tile([128, 512], F32, name=f"c0_{t}{s}", tag="ctx0")
                ctx1 = psC.tile([128, 512], F32, name=f"c1_{t}{s}", tag="ctx1")
                for i in range(NI):
                    kc, ko = i // 4, i % 4
                    sk = slice(128 * ko, 128 * ko + 128)
                    st = psS.tile([128, 1024], F32, name=f"st{t}{s}{i}", tag="st")
                    nc.tensor.matmul(
                        st[:, 0:512],
                        KT[t][kc][0:64, sk],
                        qt_lo,
                        start=True,
                        stop=True,
                        tile_position=(0, 0),
                    )
                    nc.tensor.matmul(
                        st[:, 512:1024],
                        KT[t][kc][64:128, sk],
                        qt_hi,
                        start=True,
                        stop=True,
                        tile_position=(64, 0),
                    )
                    pt = dpool.tile([128, 1024], dt_mm, name=f"pt{t}{s}{i}", tag="pt", bufs=7)
                    nc.scalar.activation(
                        pt[:], st[:], mybir.ActivationFunctionType.Exp, scale=SCALE
                    )
                    if hooks and i in hooks:
                        # deferred PE work (V tiles, next-phase projections,
                        # finished-chunk output projections) rides INSIDE the
                        # i-loop: the in-order PE stream reaches it at the
                        # ACT-paced rate, filling this chunk's PE slack
                        # without ever batching up at a phase boundary.
                        hooks[i]()
                    nc.tensor.matmul(
                        ctx0[:],
                        Vh[i][:, vb : vb + 128],
                        pt[:, 0:512],
                        start=(i == 0),
                        stop=(i == NI - 1),
                    )
                    nc.tensor.matmul(
                        ctx1[:],
                        Vh[i][:, vb + 128 : vb + 256],
                        pt[:, 512:1024],
                        start=(i == 0),
                        stop=(i == NI - 1),
                    )
                # copy out of psum promptly (frees the single ctx bank), then
                # normalize from SBUF: rows 64:128 hold the replicated
                # softmax denominator.
                cxu0 = dpool.tile([128, 512], F32, name=f"u0_{t}{s}", tag="cxu0", bufs=1)
                cxu1 = dpool.tile([128, 512], F32, name=f"u1_{t}{s}", tag="cxu1", bufs=1)
                nc.vector.tensor_copy(cxu0[:], ctx0[:])
                nc.vector.tensor_copy(cxu1[:], ctx1[:])
                rb0 = dpool.tile([64, 512], F32, name=f"rb0_{t}{s}", tag="rb0", bufs=1)
                rb1 = dpool.tile([64, 512], F32, name=f"rb1_{t}{s}", tag="rb1", bufs=1)
                if t < NT - 1:
                    nc.vector.reciprocal(rb0[:], cxu0[64:128, :])
                    nc.vector.reciprocal(rb1[:], cxu1[64:128, :])
                    # normalize on the otherwise-idle gpsimd engine
                    nc.gpsimd.tensor_mul(CX[t][s][0:64, :], cxu0[0:64, :], rb0[:])
                    nc.gpsimd.tensor_mul(CX[t][s][64:128, :], cxu1[0:64, :], rb1[:])
                else:
                    # t=3 feeds the interleaved output projection: halve the
                    # reciprocals so the CX chain completes well before the
                    # PE stream reaches the next chunk's outproj hooks; on
                    # the final chunk also push each half's outproj as soon
                    # as its 256 columns of CX are final.
                    for half in range(2):
                        cols = slice(256 * half, 256 * half + 256)
                        nc.vector.reciprocal(rb0[:, cols], cxu0[64:128, cols])
                        nc.vector.reciprocal(rb1[:, cols], cxu1[64:128, cols])
                        nc.gpsimd.tensor_mul(
                            CX[t][s][0:64, cols], cxu0[0:64, cols], rb0[:, cols]
                        )
                        nc.gpsimd.tensor_mul(
                            CX[t][s][64:128, cols], cxu1[0:64, cols], rb1[:, cols]
                        )
                        if final:
                            for q in range(4 * s + 2 * half, 4 * s + 2 * half + 2):
                                emit_outproj(q)

            PROJ_Q = (xqp_d, 0, bq_sb, QT, "q")
            PROJ_K = (xkp_d, 1024, bk_sb, KT, "k")

            # ---- warm the ACT exp table (~2.7us) during the prologue DMAs
            warm = dpool.tile([1, 16], F32, name="warm", tag="warm", bufs=1)
            nc.gpsimd.memset(warm[:], 0.0)
            nc.scalar.activation(warm[:], warm[:], mybir.ActivationFunctionType.Exp)

            # ---- prologue: one sync DMA queue, ordered by first use. The
            # first-exp path (wqk0, xq_s0, xk_s0) leads; the V path and the
            # remaining K/Q slices interleave behind it to track the
            # consumption order of chunk (0,0).
            def add_hook(hooks, i, fn):
                if i in hooks:
                    prev = hooks[i]

                    def combo(prev=prev, fn=fn):
                        prev()
                        fn()

                    hooks[i] = combo
                else:
                    hooks[i] = fn

            xv_sb = [
                [
                    spool.tile([128, 1024], dt_mm, name=f"xv{h}_{c}", tag=f"xf{h}_{c}", bufs=1)
                    for c in range(NHC)
                ]
                for h in range(2)
            ]

            nc.sync.dma_start(out=wqk_sb[0][0][:], in_=wqk_d[:, 0:1024])
            nc.sync.dma_start(out=bq_sb[:], in_=bq_d[:])
            xq00 = emit_x_dma(PROJ_Q, 0, 0)
            emit_proj(0, 0, PROJ_Q, xch=xq00)
            nc.sync.dma_start(out=wqk_sb[0][1][:], in_=wqk_d[:, 1024:2048])
            nc.sync.dma_start(out=bk_sb[:], in_=bk_d[:])
            emit_proj(0, 0, PROJ_K)
            xk_pre = {
                1: emit_x_dma(PROJ_K, 0, 1),
                2: emit_x_dma(PROJ_K, 0, 2),
                3: emit_x_dma(PROJ_K, 0, 3),
            }
            # The V-path inputs (wv, bvb, xv) ride the gpsimd software-DGE
            # queue — a second DMA path in parallel with the sync hwdge
            # queue — but gated behind the exp-critical xq(0,0) transfer so
            # the prologue's limited HBM bandwidth serves the first
            # score->exp chain before the V stream starts.
            gate = dpool.tile([1, 16], dt_mm, name="vgate", tag="vgate", bufs=1)
            nc.gpsimd.tensor_copy(gate[:], xq00[0:1, 0:16])
            nc.gpsimd.dma_start(out=wv_sb[:], in_=wvp_d[:])
            nc.gpsimd.dma_start(out=bvb_sb[:], in_=bvb_d[:])
            for h in range(2):
                for c in range(NHC):
                    nc.gpsimd.dma_start(
                        out=xv_sb[h][c][:],
                        in_=xvT_d[128 * c : 128 * c + 128, 1024 * h : 1024 * h + 1024],
                    )

            # ---- chunk (0,0): V pairs 01 jit per i-tile; K(0,s')
            # projections land where their slices arrive; Q(0,1) at the end.
            hooks = {}
            for i in range(NI):
                add_hook(hooks, i, lambda i=i: emit_v_tile(0, i))
            add_hook(hooks, 1, lambda: emit_proj(0, 1, PROJ_K, xch=xk_pre[1]))
            add_hook(hooks, 5, lambda: emit_proj(0, 2, PROJ_K, xch=xk_pre[2]))
            add_hook(hooks, 9, lambda: emit_proj(0, 3, PROJ_K, xch=xk_pre[3]))
            add_hook(hooks, 12, lambda: emit_proj(0, 1, PROJ_Q))
            emit_attention_chunk(0, 0, hooks=hooks)
            # packed weights for the later phases (queue is idle from here;
            # phase-t projections start a full phase early via hooks)
            for t in range(1, NT):
                for qk in range(2):
                    nc.sync.dma_start(
                        out=wqk_sb[t][qk][:],
                        in_=wqk_d[:, 2048 * t + 1024 * qk : 2048 * t + 1024 * qk + 1024],
                    )
            nc.sync.dma_start(out=wo_sb[:], in_=wop_d[:])

            # ---- remaining chunks: every piece of deferred PE work (next
            # s-chunk Q, next-phase Q/K projections, V pairs 23, output
            # projections at t=3) is hooked into an i-slot of a chunk whose
            # phase has ACT slack. proj_sched[(t,s)] = list of (i, fn).
            def P(t, s, which):
                return lambda: emit_proj(t, s, which)

            def VB(i):
                return lambda: emit_v_tile(1, i)

            def OP(q):
                return lambda: emit_outproj(q)

            sched = {
                (0, 1): [(2, P(1, 0, PROJ_Q)), (5, P(1, 0, PROJ_K)), (8, P(0, 2, PROJ_Q))],
                (0, 2): [(2, P(1, 1, PROJ_Q)), (5, P(1, 1, PROJ_K)), (8, P(0, 3, PROJ_Q))],
                (0, 3): [
                    (2, P(1, 2, PROJ_Q)),
                    (5, P(1, 2, PROJ_K)),
                    (9, P(1, 3, PROJ_Q)),
                    (12, P(1, 3, PROJ_K)),
                ],
                (1, 0): [(3, VB(0)), (7, VB(1)), (11, VB(2)), (13, P(2, 3, PROJ_K))],
                (1, 1): [(2, P(2, 0, PROJ_Q)), (5, P(2, 0, PROJ_K)), (8, VB(3)), (11, VB(4)), (14, VB(5))],
                (1, 2): [(2, P(2, 1, PROJ_Q)), (5, P(2, 1, PROJ_K)), (8, VB(6)), (11, VB(7)), (14, VB(8))],
                (1, 3): [(2, P(2, 2, PROJ_Q)), (5, P(2, 2, PROJ_K)), (8, VB(9)), (11, VB(10)), (14, VB(11))],
                # NOTE: KT[t][kc] tiles are sk-chunks — EVERY chunk of phase
                # t reads all four from i=12 on, so K(t,3) must land before
                # chunk (t,0) reaches i=12 (Q(t,s) is per-chunk and can lag).
                (2, 0): [(0, VB(12)), (2, VB(13)), (4, VB(14)), (6, VB(15))],
                (2, 1): [(2, P(2, 3, PROJ_Q)), (9, P(3, 0, PROJ_Q)), (12, P(3, 0, PROJ_K))],
                (2, 2): [(2, P(3, 1, PROJ_Q)), (5, P(3, 1, PROJ_K)), (9, P(3, 2, PROJ_Q)), (12, P(3, 2, PROJ_K))],
                (2, 3): [(2, P(3, 3, PROJ_Q)), (5, P(3, 3, PROJ_K))],
                (3, 1): [(9, OP(0)), (11, OP(1)), (13, OP(2)), (15, OP(3))],
                (3, 2): [(9, OP(4)), (11, OP(5)), (13, OP(6)), (15, OP(7))],
                (3, 3): [(9, OP(8)), (11, OP(9)), (13, OP(10)), (15, OP(11))],
            }

            for t in range(NT):
                for s in range(NS):
                    if t == 0 and s == 0:
                        continue
                    hooks = {}
                    for i, fn in sched.get((t, s), []):
                        add_hook(hooks, i, fn)
                    emit_attention_chunk(
                        t, s, hooks=hooks, final=(t == NT - 1 and s == NS - 1)
                    )

    return nc


def _get_nc():
    dt_mm = F32 if os.environ.get("MHA_FP32") == "1" else BF16
    key = str(dt_mm)
    if key not in _CACHED:
        _CACHED[key] = _build_nc(dt_mm)
    return _CACHED[key], dt_mm


def _pack_inputs(q_b, k_b, v_b, Wq, Wk, Wv, Wo, bq, bk, bv, rows, np_mm):
    """Build the packed per-core input map for one (batch, head-group)."""
    xqT = np.ascontiguousarray(q_b.T)  # [1024, 2048]
    xkT = np.ascontiguousarray(k_b.T)
    xvT = np.ascontiguousarray(v_b.T)
    wqT = Wq[rows, :].T  # [1024, 512]
    wkT = Wk[rows, :].T
    wvT = Wv[rows, :].T  # [1024, 512]
    woT = Wo[:, rows].T  # [512, 1024]

    def pack_x(xT):
        # [128, 4*4096]: slot (s, c) at [:, 4096*s + 512*c] = xT[128c:+128, 512s:+512]
        out = np.empty((128, 4 * 4096), dtype=np_mm)
        for s in range(4):
            for c in range(8):
                out[:, 4096 * s + 512 * c : 4096 * s + 512 * c + 512] = xT[
                    128 * c : 128 * c + 128, 512 * s : 512 * s + 512
                ]
        return out

    wqk = np.empty((128, 4 * 2048), dtype=np_mm)
    for t in range(4):
        for c in range(8):
            wqk[:, 2048 * t + 128 * c : 2048 * t + 128 * c + 128] = wqT[
                128 * c : 128 * c + 128, 128 * t : 128 * t + 128
            ]
            wqk[:, 2048 * t + 1024 + 128 * c : 2048 * t + 1024 + 128 * c + 128] = wkT[
                128 * c : 128 * c + 128, 128 * t : 128 * t + 128
            ]
    wvp = np.empty((128, 4096), dtype=np_mm)
    for c in range(8):
        wvp[:, 512 * c : 512 * c + 512] = wvT[128 * c : 128 * c + 128, :]
    wop = np.empty((128, 4096), dtype=np_mm)
    for t in range(4):
        wop[:, 1024 * t : 1024 * t + 1024] = woT[128 * t : 128 * t + 128, :]

    return {
        "xqp": pack_x(xqT),
        "xkp": pack_x(xkT),
        "xvT": xvT.astype(np_mm),
        "wqk": wqk,
        "wvp": wvp,
        "wop": wop,
        "bq2": np.ascontiguousarray(bq[rows].reshape(4, 128).T),
        "bk2": np.ascontiguousarray(bk[rows].reshape(4, 128).T),
        "bvb": np.ascontiguousarray(np.broadcast_to(bv[rows], (128, HL))),
    }


def kernel(query, key, value, Wq, bq, Wk, bk, Wv, bv, Wo, bo):
    nc, dt_mm = _get_nc()
    np_mm = ml_dtypes.bfloat16 if dt_mm == BF16 else np.float32

    query = np.asarray(query, dtype=np.float32)
    key = np.asarray(key, dtype=np.float32)
    value = np.asarray(value, dtype=np.float32)
    Wq = np.asarray(Wq, dtype=np.float32)
    Wk = np.asarray(Wk, dtype=np.float32)
    Wv = np.asarray(Wv, dtype=np.float32)
    Wo = np.asarray(Wo, dtype=np.float32)
    bq = np.asarray(bq, dtype=np.float32)
    bk = np.asarray(bk, dtype=np.float32)
    bv = np.asarray(bv, dtype=np.float32)
    bo = np.asarray(bo, dtype=np.float32)

    in_maps = []
    for c in range(NCORES):
        b_idx, hg = c // 2, c % 2
        rows = slice(HL * hg, HL * hg + HL)
        m = _pack_inputs(
            query[b_idx].astype(np_mm),
            key[b_idx].astype(np_mm),
            value[b_idx].astype(np_mm),
            Wq.astype(np_mm),
            Wk.astype(np_mm),
            Wv.astype(np_mm),
            Wo.astype(np_mm),
            bq,
            bk,
            bv,
            rows,
            np_mm,
        )
        in_maps.append(m)

    trace = os.environ.get("MHA_TRACE") == "1"
    res = run_bass_kernel_spmd(nc, in_maps, list(range(NCORES)), trace=trace)
    if trace:
        kernel.last_exec_time_ns = res.exec_time_ns
        kernel.last_results = res

    out = np.empty((B, SQ, HIDDEN), dtype=np.float32)
    for b_idx in range(B):
        out[b_idx] = res.results[2 * b_idx]["out"]
        out[b_idx] += res.results[2 * b_idx + 1]["out"]
    out += bo[None, None, :]
    return out


# revision 35
# speedup vs baseline: 1.0271x; 1.0271x over previous
"""Multi-head attention (B=4, S=2048, H=1024, 16 heads) on 8 trn2 NeuronCores.

Sharding: data-parallel over batch (4) x tensor-parallel over head-groups (2):
core c handles batch c//2, heads 8*(c%2) .. 8*(c%2)+8. Each core computes its
partial output projection; host sums the two head-group partials + bo.

Per-core device algorithm (all matmul inputs bf16, fp32 accumulation):
  inputs are pre-transposed/packed on host: xqp/xkp hold x^T slices laid out
  so each (t,s) projection needs ONE [128,4096] DMA; wqk packs the per-
  head-pair Q and K weight slices into one [128,2048] DMA per phase.
  QT[t] (128=2 heads' d, sq) = wqk-slices^T @ x-chunks (+bias)
  KT[t][s'] likewise (split per sk-chunk for fine-grained deps)
  V halves (pairs 01 / 23), per 128-sk tile: [values(64)|ones(64)] per head;
  the ones columns make the ctx matmul emit the softmax denominator
  replicated in psum rows 64:128 at zero PE cost (matmul time ~ N only).
  per head-pair t, sq-chunk s (512), sk-tile i (128):
     S^T = KT-slice^T @ QT-slice  (two heads row-packed, run concurrently)
     P^T = exp(S^T * 0.125)       (ACT, psum->sbuf, bf16 out)
     ctx (128, 512) += V-block^T @ P^T
  normalize: ctx psum -> sbuf copy, reciprocal of rows 64:128 (DVE),
     CX = ctx * recip (gpsimd, to keep DVE short)
  out (sq, 1024) = sum_t CX-chunks^T @ woT   -> DMA out (fp32)

Schedule: ACT (exp) is the hard floor (256 x ~1.15us). Phase t=0 pays the
V-pair-01 projection jit inside chunk (0,0) and interleaves K(0,s')
projections with DMA arrival; V pairs 23 and the t+1 Q/K projections hide
in the ACT-bound phases' PE slack; the output projection of s-chunk j is
emitted at the end of chunk (3, j+1) so the in-order PE stream reaches it
only after the DVE epilogue that finalizes CX has long finished.
"""
import os
import sys

sys.path.insert(0, "/opt/trn_rl_repo")

import numpy as np
import ml_dtypes

import concourse.bass as bass
import concourse.mybir as mybir
import concourse.tile as tile

# ---------------------------------------------------------------------------
# Walrus in this environment allows at most 1 sync wait per instruction (2 for
# EventSemaphore); Tile sometimes emits more (e.g. the exit drain). Hoist the
# extra waits onto EventSemaphore instructions inserted before the offender.
import json as _json


def _transform_bir_json(bir_bytes: bytes) -> bytes:
    bir = _json.loads(bir_bytes)
    changed = False
    ctr = 0
    for fn in bir.get("functions", []):
        for blk in fn.get("blocks", []):
            out = []
            for inst in blk.get("instructions", []):
                si = inst.get("sync_info") or {}
                waits = si.get("on_wait") or []
                cap = 2 if inst.get("opcode") == "EventSemaphore" else 1
                if len(waits) > cap:
                    changed = True
                    extra = waits[:-cap]
                    si["on_wait"] = waits[-cap:]
                    for i in range(0, len(extra), 2):
                        ctr += 1
                        out.append(
                            {
                                "debug": inst.get("debug"),
                                "engine": inst["engine"],
                                "ins": [],
                                "name": f"{inst['name']}_xw{ctr}",
                                "opcode": "EventSemaphore",
                                "outs": [],
                                "sync_info": {
                                    "on_update": [],
                                    "on_wait": extra[i : i + 2],
                                },
                            }
                        )
                out.append(inst)
            blk["instructions"] = out
    if not changed:
        return bir_bytes
    return _json.dumps(bir).encode()


def _apply_bir_patch():
    import concourse.bass_utils as bu
    import concourse.bass2jax as b2j

    if getattr(b2j, "_bir_waitfix_applied", False):
        return
    orig = bu.compile_bir_kernel

    def patched(bir_json, tmpdir, neff_name="file.neff"):
        return orig(_transform_bir_json(bir_json), tmpdir, neff_name)

    b2j.compile_bir_kernel = patched
    bu.compile_bir_kernel = patched
    b2j._bir_waitfix_applied = True


_apply_bir_patch()

from concourse.bass_utils import run_bass_kernel_spmd  # noqa: E402

# ---------------------------------------------------------------------------
HIDDEN = 1024
HEADS = 16
HD = 64  # head dim
B, SQ, SK = 4, 2048, 2048
NCORES = 8
HPC = 8  # heads per core (tensor-parallel over 2 head groups)
HL = HPC * HD  # local hidden slice = 512
SCALE = HD ** -0.5

F32 = mybir.dt.float32
BF16 = mybir.dt.bfloat16

_CACHED = {}


def _build_nc(dt_mm):
    nc = bass.Bass()
    xqp_d = nc.declare_dram_parameter("xqp", [128, 4 * 4096], dt_mm, isOutput=False)
    xkp_d = nc.declare_dram_parameter("xkp", [128, 4 * 4096], dt_mm, isOutput=False)
    xvT_d = nc.declare_dram_parameter("xvT", [HIDDEN, SK], dt_mm, isOutput=False)
    wqk_d = nc.declare_dram_parameter("wqk", [128, 4 * 2048], dt_mm, isOutput=False)
    wvp_d = nc.declare_dram_parameter("wvp", [128, 4096], dt_mm, isOutput=False)
    wop_d = nc.declare_dram_parameter("wop", [128, 4096], dt_mm, isOutput=False)
    bq_d = nc.declare_dram_parameter("bq2", [128, 4], F32, isOutput=False)
    bk_d = nc.declare_dram_parameter("bk2", [128, 4], F32, isOutput=False)
    bvb_d = nc.declare_dram_parameter("bvb", [128, HL], F32, isOutput=False)
    out_d = nc.declare_dram_parameter("out", [SQ, HIDDEN], F32, isOutput=True)

    NHC = HIDDEN // 128  # 8 hidden chunks
    NT = 4  # head-pair tiles (8 local heads -> 4 pairs of 64 rows)
    NS = 4  # sq chunks of 512
    NI = SK // 128  # 16 sk tiles

    with tile.TileContext(nc) as tc:
        from contextlib import ExitStack

        with ExitStack() as stack:
            wpool = stack.enter_context(tc.tile_pool(name="wpool", bufs=1))
            apool = stack.enter_context(tc.tile_pool(name="apool", bufs=1))

            # ---- persistent weights / biases (DMAs emitted at point of need)
            # Q and K weight slices as separate tiles so the very first Q
            # projection doesn't wait on the K half of the transfer.
            wqk_sb = [
                [
                    wpool.tile([128, 1024], dt_mm, name=f"w{qk}{t}", tag=f"w{qk}{t}")
                    for qk in ("q", "k")
                ]
                for t in range(NT)
            ]
            wv_sb = wpool.tile([128, 4096], dt_mm, name="wvp", tag="wvp")
            wo_sb = wpool.tile([128, 4096], dt_mm, name="wop", tag="wop")
            bq_sb = wpool.tile([128, 4], F32)
            bk_sb = wpool.tile([128, 4], F32)
            bvb_sb = wpool.tile([128, HL], F32)

            # ---- persistent activations. All split per (t, s-chunk) so
            # every dependency is tile-granular: projections, epilogues and
            # deferred work can interleave into the attention chunks without
            # false write-during-read hazards.
            QT = [
                [
                    apool.tile([128, 512], dt_mm, name=f"QT{t}_{s}", tag=f"QT{t}_{s}")
                    for s in range(NS)
                ]
                for t in range(NT)
            ]
            KT = [
                [
                    apool.tile([128, 512], dt_mm, name=f"KT{t}_{c}", tag=f"KT{t}_{c}")
                    for c in range(NS)
                ]
                for t in range(NT)
            ]
            # V in two halves (head pairs 01 / 23), [128, 512] each:
            # per head a 128-col block [0:64]=values, [64:128]=ones.
            VH = [
                [
                    apool.tile([128, 512], dt_mm, name=f"V{h}_{i}", tag=f"V{h}_{i}")
                    for i in range(NI)
                ]
                for h in range(2)
            ]
            CX = [
                [
                    apool.tile([128, 512], dt_mm, name=f"CX{t}_{s}", tag=f"CX{t}_{s}")
                    for s in range(NS)
                ]
                for t in range(NT)
            ]

            inner = stack.enter_context(ExitStack())
            spool = inner.enter_context(tc.tile_pool(name="ldpool", bufs=2))
            dpool = inner.enter_context(tc.tile_pool(name="dpool", bufs=4))
            psA = inner.enter_context(tc.tile_pool(name="psA", bufs=2, space="PSUM"))
            psS = inner.enter_context(tc.tile_pool(name="psS", bufs=2, space="PSUM"))
            psC = inner.enter_context(tc.tile_pool(name="psC", bufs=1, space="PSUM"))

            def emit_x_dma(which, t, s):
                xp_d, off, b_sb, OUT, nm = which
                xch = spool.tile(
                    [128, 4096], dt_mm, name=f"x{nm}{t}{s}", tag="xch", bufs=3
                )
                nc.sync.dma_start(
                    out=xch[:], in_=xp_d[:, 4096 * s : 4096 * s + 4096]
                )
                return xch

            def emit_proj(t, s, which, xch=None):
                xp_d, off, b_sb, OUT, nm = which
                if xch is None:
                    xch = emit_x_dma(which, t, s)
                ps = psA.tile([128, 512], F32, name=f"ps{nm}{s}{t}", tag="psA")
                w = wqk_sb[t][0 if off == 0 else 1]
                for c in range(NHC):
                    nc.tensor.matmul(
                        ps[:],
                        w[:, 128 * c : 128 * c + 128],
                        xch[:, 512 * c : 512 * c + 512],
                        start=(c == 0),
                        stop=(c == NHC - 1),
                    )
                nc.vector.tensor_scalar_add(OUT[t][s][:, :], ps[:], b_sb[:, t : t + 1])

            def emit_v_tile(h, i):
                # V half h (head pairs 2h,2h+1), sk-tile i: 8 accumulating
                # matmuls of N=256 into half a psA slot, then ones + bias.
                # xv is split in sk-halves so the i<8 tiles only wait on the
                # first half of the (late-arriving) xv DMA stream.
                xvh = xv_sb[i // 8]
                io = 128 * (i % 8)
                ps = psA.tile([128, 512], F32, name=f"psv{h}_{i}", tag="psA")
                for c in range(NHC):
                    nc.tensor.matmul(
                        ps[:, 0:256],
                        xvh[c][:, io : io + 128],
                        wv_sb[:, 512 * c + 256 * h : 512 * c + 256 * h + 256],
                        start=(c == 0),
                        stop=(c == NHC - 1),
                    )
                nc.gpsimd.memset(VH[h][i][:], 1.0)
                vv = VH[h][i].rearrange("p (h e) -> p h e", e=128)
                nc.vector.tensor_add(
                    vv[:, :, 0:HD],
                    ps[:, 0:256].rearrange("p (h d) -> p h d", d=HD),
                    bvb_sb[:, 256 * h : 256 * h + 256].rearrange("p (h d) -> p h d", d=HD),
                )

            def emit_outproj(q):
                # output projection for one finished q-tile; reuses the psA
                # slots that the (by now finished) projections vacated.
                qs, qo = q // 4, 128 * (q % 4)
                ot = dpool.tile([128, HIDDEN], F32, name=f"ot{q}", tag="ot", bufs=2)
                for half in range(2):
                    po = psA.tile([128, 512], F32, name=f"po{q}_{half}", tag="psA")
                    for tt in range(NT):
                        nc.tensor.matmul(
                            po[:],
                            CX[tt][qs][:, qo : qo + 128],
                            wo_sb[:, 1024 * tt + 512 * half : 1024 * tt + 512 * half + 512],
                            start=(tt == 0),
                            stop=(tt == NT - 1),
                        )
                    nc.vector.tensor_copy(ot[:, 512 * half : 512 * half + 512], po[:])
                nc.sync.dma_start(out=out_d[128 * q : 128 * q + 128, :], in_=ot[:])

            def emit_attention_chunk(t, s, hooks=None, final=False):
                qt_lo, qt_hi = QT[t][s][0:64, :], QT[t][s][64:128, :]
                Vh = VH[t // 2]
                vb = 256 * (t % 2)  # pair block offset inside the half
                ctx0 = psC.tile([128, 512], F32, name=f"c0_{t}{s}", tag="ctx0")
                ctx1 = psC.tile([128, 512], F32, name=f"c1_{t}{s}", tag="ctx1")
                for i in range(NI):
                    kc, ko = i // 4, i % 4
                    sk = slice(128 * ko, 128 * ko + 128)
                    st = psS.tile([128, 1024], F32, name=f"st{t}{s}{i}", tag="st")
                    nc.tensor.matmul(
                        st[:, 0:512],
                        KT[t][kc][0:64, sk],
                        qt_lo,
                        start=True,
                        stop=True,
                        tile_position=(0, 0),
                    )
                    nc.tensor.matmul(
                        st[:, 512:1024],
                        KT[t][kc][64:128, sk],
                        qt_hi,
                        start=True,
                        stop=True,
                        tile_position=(64, 0),
                    )
                    pt = dpool.tile([128, 1024], dt_mm, name=f"pt{t}{s}{i}", tag="pt", bufs=7)
                    nc.scalar.activation(
                        pt[:], st[:], mybir.ActivationFunctionType.Exp, scale=SCALE
                    )
                    if hooks and i in hooks:
                        # deferred PE work (V tiles, next-phase projections,
                        # finished-chunk output projections) rides INSIDE the
                        # i-loop: the in-order PE stream reaches it at the
                        # ACT-paced rate, filling this chunk's PE slack
                        # without ever batching up at a phase boundary.
                        hooks[i]()
                    nc.tensor.matmul(
                        ctx0[:],
                        Vh[i][:, vb : vb + 128],
                        pt[:, 0:512],
                        start=(i == 0),
                        stop=(i == NI - 1),
                    )
                    nc.tensor.matmul(
                        ctx1[:],
                        Vh[i][:, vb + 128 : vb + 256],
                        pt[:, 512:1024],
                        start=(i == 0),
                        stop=(i == NI - 1),
                    )
                # copy out of psum promptly (frees the single ctx bank), then
                # normalize from SBUF: rows 64:128 hold the replicated
                # softmax denominator.
                cxu0 = dpool.tile([128, 512], F32, name=f"u0_{t}{s}", tag="cxu0", bufs=1)
                cxu1 = dpool.tile([128, 512], F32, name=f"u1_{t}{s}", tag="cxu1", bufs=1)
                nc.vector.tensor_copy(cxu0[:], ctx0[:])
                nc.vector.tensor_copy(cxu1[:], ctx1[:])
                rb0 = dpool.tile([64, 512], F32, name=f"rb0_{t}{s}", tag="rb0", bufs=1)
                rb1 = dpool.tile([64, 512], F32, name=f"rb1_{t}{s}", tag="rb1", bufs=1)
                if t < NT - 1:
                    nc.vector.reciprocal(rb0[:], cxu0[64:128, :])
                    nc.vector.reciprocal(rb1[:], cxu1[64:128, :])
                    # normalize on the otherwise-idle gpsimd engine
                    nc.gpsimd.tensor_mul(CX[t][s][0:64, :], cxu0[0:64, :], rb0[:])
                    nc.gpsimd.tensor_mul(CX[t][s][64:128, :], cxu1[0:64, :], rb1[:])
                else:
                    # t=3 feeds the interleaved output projection: halve the
                    # reciprocals so the CX chain completes well before the
                    # PE stream reaches the next chunk's outproj hooks; on
                    # the final chunk also push each half's outproj as soon
                    # as its 256 columns of CX are final.
                    for half in range(2):
                        cols = slice(256 * half, 256 * half + 256)
                        nc.vector.reciprocal(rb0[:, cols], cxu0[64:128, cols])
                        nc.vector.reciprocal(rb1[:, cols], cxu1[64:128, cols])
                        nc.gpsimd.tensor_mul(
                            CX[t][s][0:64, cols], cxu0[0:64, cols], rb0[:, cols]
                        )
                        nc.gpsimd.tensor_mul(
                            CX[t][s][64:128, cols], cxu1[0:64, cols], rb1[:, cols]
                        )
                        if final:
                            for q in range(4 * s + 2 * half, 4 * s + 2 * half + 2):
                                emit_outproj(q)

            PROJ_Q = (xqp_d, 0, bq_sb, QT, "q")
            PROJ_K = (xkp_d, 1024, bk_sb, KT, "k")

            # ---- warm the ACT exp table (~2.7us) during the prologue DMAs
            warm = dpool.tile([1, 16], F32, name="warm", tag="warm", bufs=1)
            nc.gpsimd.memset(warm[:], 0.0)
            nc.scalar.activation(warm[:], warm[:], mybir.ActivationFunctionType.Exp)

            # ---- prologue: one sync DMA queue, ordered by first use. The
            # first-exp path (wqk0, xq_s0, xk_s0) leads; the V path and the
            # remaining K/Q slices interleave behind it to track the
            # consumption order of chunk (0,0).
            def add_hook(hooks, i, fn):
                if i in hooks:
                    prev = hooks[i]

                    def combo(prev=prev, fn=fn):
                        prev()
                        fn()

                    hooks[i] = combo
                else:
                    hooks[i] = fn

            xv_sb = [
                [
                    spool.tile([128, 1024], dt_mm, name=f"xv{h}_{c}", tag=f"xf{h}_{c}", bufs=1)
                    for c in range(NHC)
                ]
                for h in range(2)
            ]

            nc.sync.dma_start(out=wqk_sb[0][0][:], in_=wqk_d[:, 0:1024])
            nc.sync.dma_start(out=bq_sb[:], in_=bq_d[:])
            xq00 = emit_x_dma(PROJ_Q, 0, 0)
            emit_proj(0, 0, PROJ_Q, xch=xq00)
            nc.sync.dma_start(out=wqk_sb[0][1][:], in_=wqk_d[:, 1024:2048])
            nc.sync.dma_start(out=bk_sb[:], in_=bk_d[:])
            emit_proj(0, 0, PROJ_K)
            # sync-queue order tracks chunk (0,0)'s consumption: wv + the
            # first sk-half of xv (V tiles 0..7) behind the exp-critical
            # slices, xk_s1 for scores i>=4, then the second xv half.
            nc.sync.dma_start(out=wv_sb[:], in_=wvp_d[:])
            nc.sync.dma_start(out=bvb_sb[:], in_=bvb_d[:])
            xk_pre = {1: emit_x_dma(PROJ_K, 0, 1)}
            for c in range(NHC):
                nc.sync.dma_start(
                    out=xv_sb[0][c][:], in_=xvT_d[128 * c : 128 * c + 128, 0:1024]
                )
            xk_pre[2] = emit_x_dma(PROJ_K, 0, 2)
            for c in range(NHC):
                nc.sync.dma_start(
                    out=xv_sb[1][c][:], in_=xvT_d[128 * c : 128 * c + 128, 1024:2048]
                )
            xk_pre[3] = emit_x_dma(PROJ_K, 0, 3)

            # ---- chunk (0,0): V pairs 01 jit per i-tile; K(0,s')
            # projections land where their slices arrive; Q(0,1) at the end.
            hooks = {}
            for i in range(NI):
                add_hook(hooks, i, lambda i=i: emit_v_tile(0, i))
            add_hook(hooks, 1, lambda: emit_proj(0, 1, PROJ_K, xch=xk_pre[1]))
            add_hook(hooks, 5, lambda: emit_proj(0, 2, PROJ_K, xch=xk_pre[2]))
            add_hook(hooks, 9, lambda: emit_proj(0, 3, PROJ_K, xch=xk_pre[3]))
            add_hook(hooks, 12, lambda: emit_proj(0, 1, PROJ_Q))
            emit_attention_chunk(0, 0, hooks=hooks)
            # packed weights for the later phases (queue is idle from here;
            # phase-t projections start a full phase early via hooks)
            for t in range(1, NT):
                for qk in range(2):
                    nc.sync.dma_start(
                        out=wqk_sb[t][qk][:],
                        in_=wqk_d[:, 2048 * t + 1024 * qk : 2048 * t + 1024 * qk + 1024],
                    )
            nc.sync.dma_start(out=wo_sb[:], in_=wop_d[:])

            # ---- remaining chunks: every piece of deferred PE work (next
            # s-chunk Q, next-phase Q/K projections, V pairs 23, output
            # projections at t=3) is hooked into an i-slot of a chunk whose
            # phase has ACT slack. proj_sched[(t,s)] = list of (i, fn).
            def P(t, s, which):
                return lambda: emit_proj(t, s, which)

            def VB(i):
                return lambda: emit_v_tile(1, i)

            def OP(q):
                return lambda: emit_outproj(q)

            sched = {
                (0, 1): [(2, P(1, 0, PROJ_Q)), (5, P(1, 0, PROJ_K)), (8, P(0, 2, PROJ_Q))],
                (0, 2): [(2, P(1, 1, PROJ_Q)), (5, P(1, 1, PROJ_K)), (8, P(0, 3, PROJ_Q))],
                (0, 3): [
                    (2, P(1, 2, PROJ_Q)),
                    (5, P(1, 2, PROJ_K)),
                    (9, P(1, 3, PROJ_Q)),
                    (12, P(1, 3, PROJ_K)),
                ],
                (1, 0): [(3, VB(0)), (7, VB(1)), (11, VB(2)), (13, P(2, 3, PROJ_K))],
                (1, 1): [(2, P(2, 0, PROJ_Q)), (5, P(2, 0, PROJ_K)), (8, VB(3)), (11, VB(4)), (14, VB(5))],
                (1, 2): [(2, P(2, 1, PROJ_Q)), (5, P(2, 1, PROJ_K)), (8, VB(6)), (11, VB(7)), (14, VB(8))],
                (1, 3): [(2, P(2, 2, PROJ_Q)), (5, P(2, 2, PROJ_K)), (8, VB(9)), (11, VB(10)), (14, VB(11))],
                # NOTE: KT[t][kc] tiles are sk-chunks — EVERY chunk of phase
                # t reads all four from i=12 on, so K(t,3) must land before
                # chunk (t,0) reaches i=12 (Q(t,s) is per-chunk and can lag).
                (2, 0): [(0, VB(12)), (2, VB(13)), (4, VB(14)), (6, VB(15))],
                (2, 1): [(2, P(2, 3, PROJ_Q)), (9, P(3, 0, PROJ_Q)), (12, P(3, 0, PROJ_K))],
                (2, 2): [(2, P(3, 1, PROJ_Q)), (5, P(3, 1, PROJ_K)), (9, P(3, 2, PROJ_Q)), (12, P(3, 2, PROJ_K))],
                (2, 3): [(2, P(3, 3, PROJ_Q)), (5, P(3, 3, PROJ_K))],
                (3, 1): [(9, OP(0)), (11, OP(1)), (13, OP(2)), (15, OP(3))],
                (3, 2): [(9, OP(4)), (11, OP(5)), (13, OP(6)), (15, OP(7))],
                (3, 3): [(9, OP(8)), (11, OP(9)), (13, OP(10)), (15, OP(11))],
            }

            for t in range(NT):
                for s in range(NS):
                    if t == 0 and s == 0:
                        continue
                    hooks = {}
                    for i, fn in sched.get((t, s), []):
                        add_hook(hooks, i, fn)
                    emit_attention_chunk(
                        t, s, hooks=hooks, final=(t == NT - 1 and s == NS - 1)
                    )

    return nc


def _get_nc():
    dt_mm = F32 if os.environ.get("MHA_FP32") == "1" else BF16
    key = str(dt_mm)
    if key not in _CACHED:
        _CACHED[key] = _build_nc(dt_mm)
    return _CACHED[key], dt_mm


def _pack_inputs(q_b, k_b, v_b, Wq, Wk, Wv, Wo, bq, bk, bv, rows, np_mm):
    """Build the packed per-core input map for one (batch, head-group)."""
    xqT = np.ascontiguousarray(q_b.T)  # [1024, 2048]
    xkT = np.ascontiguousarray(k_b.T)
    xvT = np.ascontiguousarray(v_b.T)
    wqT = Wq[rows, :].T  # [1024, 512]
    wkT = Wk[rows, :].T
    wvT = Wv[rows, :].T  # [1024, 512]
    woT = Wo[:, rows].T  # [512, 1024]

    def pack_x(xT):
        # [128, 4*4096]: slot (s, c) at [:, 4096*s + 512*c] = xT[128c:+128, 512s:+512]
        out = np.empty((128, 4 * 4096), dtype=np_mm)
        for s in range(4):
            for c in range(8):
                out[:, 4096 * s + 512 * c : 4096 * s + 512 * c + 512] = xT[
                    128 * c : 128 * c + 128, 512 * s : 512 * s + 512
                ]
        return out

    wqk = np.empty((128, 4 * 2048), dtype=np_mm)
    for t in range(4):
        for c in range(8):
            wqk[:, 2048 * t + 128 * c : 2048 * t + 128 * c + 128] = wqT[
                128 * c : 128 * c + 128, 128 * t : 128 * t + 128
            ]
            wqk[:, 2048 * t + 1024 + 128 * c : 2048 * t + 1024 + 128 * c + 128] = wkT[
                128 * c : 128 * c + 128, 128 * t : 128 * t + 128
            ]
    wvp = np.empty((128, 4096), dtype=np_mm)
    for c in range(8):
        wvp[:, 512 * c : 512 * c + 512] = wvT[128 * c : 128 * c + 128, :]
    wop = np.empty((128, 4096), dtype=np_mm)
    for t in range(4):
        wop[:, 1024 * t : 1024 * t + 1024] = woT[128 * t : 128 * t + 128, :]

    return {
        "xqp": pack_x(xqT),
        "xkp": pack_x(xkT),
        "xvT": xvT.astype(np_mm),
        "wqk": wqk,
        "wvp": wvp,
        "wop": wop,
        "bq2": np.ascontiguousarray(bq[rows].reshape(4, 128).T),
        "bk2": np.ascontiguousarray(bk[rows].reshape(4, 128).T),
        "bvb": np.ascontiguousarray(np.broadcast_to(bv[rows], (128, HL))),
    }


def kernel(query, key, value, Wq, bq, Wk, bk, Wv, bv, Wo, bo):
    nc, dt_mm = _get_nc()
    np_mm = ml_dtypes.bfloat16 if dt_mm == BF16 else np.float32

    query = np.asarray(query, dtype=np.float32)
    key = np.asarray(key, dtype=np.float32)
    value = np.asarray(value, dtype=np.float32)
    Wq = np.asarray(Wq, dtype=np.float32)
    Wk = np.asarray(Wk, dtype=np.float32)
    Wv = np.asarray(Wv, dtype=np.float32)
    Wo = np.asarray(Wo, dtype=np.float32)
    bq = np.asarray(bq, dtype=np.float32)
    bk = np.asarray(bk, dtype=np.float32)
    bv = np.asarray(bv, dtype=np.float32)
    bo = np.asarray(bo, dtype=np.float32)

    in_maps = []
    for c in range(NCORES):
        b_idx, hg = c // 2, c % 2
        rows = slice(HL * hg, HL * hg + HL)
        m = _pack_inputs(
            query[b_idx].astype(np_mm),
            key[b_idx].astype(np_mm),
            value[b_idx].astype(np_mm),
            Wq.astype(np_mm),
            Wk.astype(np_mm),
            Wv.astype(np_mm),
            Wo.astype(np_mm),
            bq,
            bk,
            bv,
            rows,
            np_mm,
        )
        in_maps.append(m)

    trace = os.environ.get("MHA_TRACE") == "1"
    res = run_bass_kernel_spmd(nc, in_maps, list(range(NCORES)), trace=trace)
    if trace:
        kernel.last_exec_time_ns = res.exec_time_ns
        kernel.last_results = res

    out = np.empty((B, SQ, HIDDEN), dtype=np.float32)
    for b_idx in range(B):
        out[b_idx] = res.results[2 * b_idx]["out"]
        out[b_idx] += res.results[2 * b_idx + 1]["out"]
    out += bo[None, None, :]
    return out
